# revision 8
# baseline (speedup 1.0000x reference)
"""Vision-RWKV (RWKV-v4 spatial mix) encoder block on 8 Trainium2 NeuronCores.

Strategy: data-parallel over batch B=16 -> 2 batches per core, no collectives.
Layout: channel-major [c, t] on-chip. The WKV recurrence runs as a hardware
tensor_tensor_scan along the free (token) dim in fp16 (fp32 internal state).

Projections run as fp8e4 DoubleRow matmuls (2 contraction blocks per pass,
0.5 cyc/row). Precision is recovered with split operands, all sharing one
x64 product scale so they accumulate in a single PSUM group:
  k, r: W_hi + W_lo   (2 terms)
  v:    W_hi@x_hi + W_lo@x_hi + W_hi@x_lo   (3 terms)
  o:    same 3-term with on-chip hi/lo split of the gated activation
Mixed activations (mix_* lerps of x and its spatial shift) are host-computed
and shipped pre-quantized to fp8. Gating uses tanh (sigmoid rewritten as
0.5*(tanh(r/2)+1) with the 0.5 folded into the LN affine) so every ACT
function except the per-chunk Sqrt lives in one activation table.

Self-contained: hardcodes B=16, T=1024, C=1024, H=W=32, 8 cores.
"""
import sys
sys.path.insert(0, "/opt/trn_rl_repo")

from contextlib import ExitStack

import numpy as np
import ml_dtypes

import concourse.bacc as bacc
import concourse.tile as tile
from concourse import mybir
from concourse.bass_utils import run_bass_kernel_spmd

dt = mybir.dt
AF = mybir.ActivationFunctionType
ALU = mybir.AluOpType

B, T, C = 16, 1024, 1024
NCORES = 8
BL = B // NCORES          # batches per core
NCT = C // 128            # channel tiles (8)
TC = 512                  # token chunk
NCH = T // TC             # chunks per batch (2)
NTT = TC // 128           # token tiles per chunk (4)
NPAIR = NCT // 2          # DoubleRow contraction pairs (4)
EPS = 1e-5
WS = 64.0                 # weight quant scale; PSUM holds 64*(W@x)

F32, F16, F8 = dt.float32, dt.float16, dt.float8e4
E4 = ml_dtypes.float8_e4m3fn
DR = mybir.MatmulPerfMode.DoubleRow

# cvec column offsets
EW, EU, LW2, LB2 = 0, 8, 16, 24
NVEC = 32

_CACHE = {}


def build():
    nc = bacc.Bacc("TRN2", target_bir_lowering=False, debug=False)

    mk_d = nc.dram_tensor("mk", [BL, C, T], F8, kind="ExternalInput")
    mr_d = nc.dram_tensor("mr", [BL, C, T], F8, kind="ExternalInput")
    mvh_d = nc.dram_tensor("mvh", [BL, C, T], F8, kind="ExternalInput")
    mvl_d = nc.dram_tensor("mvl", [BL, C, T], F8, kind="ExternalInput")
    w_names = ["wkh", "wkl", "wvh", "wvl", "wrh", "wrl"]
    w_drams = {n: nc.dram_tensor(n, [NCT, C, 128], F8, kind="ExternalInput")
               for n in w_names}
    woh_d = nc.dram_tensor("woh", [C, C], F8, kind="ExternalInput")
    wol_d = nc.dram_tensor("wol", [C, C], F8, kind="ExternalInput")
    cvec_d = nc.dram_tensor("cvec", [128, NVEC], F32, kind="ExternalInput")
    out_d = nc.dram_tensor("out", [BL, T, C], F32, kind="ExternalOutput")

    with tile.TileContext(nc) as tc, ExitStack() as ctx:
        cons = ctx.enter_context(tc.tile_pool(name="cons", bufs=1))
        wp = ctx.enter_context(tc.tile_pool(name="wp", bufs=1))
        mxp = ctx.enter_context(tc.tile_pool(name="mxp", bufs=2))
        ekp = ctx.enter_context(tc.tile_pool(name="ekp", bufs=2))
        pqp = ctx.enter_context(tc.tile_pool(name="pqp", bufs=2))
        ndp = ctx.enter_context(tc.tile_pool(name="ndp", bufs=2))
        ypl = ctx.enter_context(tc.tile_pool(name="ypl", bufs=1))
        ysq = ctx.enter_context(tc.tile_pool(name="ysq", bufs=2))
        stp = ctx.enter_context(tc.tile_pool(name="stp", bufs=2))
        gtp = ctx.enter_context(tc.tile_pool(name="gtp", bufs=6))
        thp = ctx.enter_context(tc.tile_pool(name="thp", bufs=2))
        syp = ctx.enter_context(tc.tile_pool(name="syp", bufs=2))
        osg = ctx.enter_context(tc.tile_pool(name="osg", bufs=2))
        carp = ctx.enter_context(tc.tile_pool(name="carp", bufs=1))

        pp = ctx.enter_context(tc.tile_pool(name="pp", bufs=4, space="PSUM"))
        sps = ctx.enter_context(tc.tile_pool(name="sps", bufs=1, space="PSUM"))
        opl = ctx.enter_context(tc.tile_pool(name="opl", bufs=2, space="PSUM"))

        # ---- constants ----
        cvec = cons.tile([128, NVEC], F32)
        nc.sync.dma_start(cvec[:], cvec_d.ap()[:])
        ones16 = cons.tile([128, 1], F16)
        nc.vector.memset(ones16[:], 1.0 / C)
        eps_t = cons.tile([1, 1], F32)
        nc.vector.memset(eps_t[:], EPS)
        one_t = cons.tile([128, 1], F32)
        nc.vector.memset(one_t[:], 1.0)

        # PE warmup while initial DMAs land
        wu = cons.tile([128, 128], F16)
        nc.vector.memset(wu[:], 0.5)
        wu_ps = pp.tile([128, 128], F32, tag="proj")
        for _ in range(60):
            nc.tensor.matmul(wu_ps[:], wu[:], wu[:], start=True, stop=True)

        # packed fp16 e^w rows per j-block (keeps the scans in DVE 2x mode)
        ew16 = cons.tile([128, NCT * TC], F16)
        for j in range(NCT):
            nc.scalar.activation(
                ew16[:, j * TC:(j + 1) * TC],
                cvec[:, EW + j:EW + j + 1].broadcast_to([128, TC]), AF.Copy)

        # ---- weights (fp8 hi/lo pairs, j-major pair layout) ----
        wsb = {n: wp.tile([128, NCT * C], F8, tag=n, name=n) for n in w_names}
        woh = wp.tile([128, NCT * C], F8, tag="woh")
        wol = wp.tile([128, NCT * C], F8, tag="wol")
        for n in w_names:
            eng = nc.scalar if n[1] in "kr" else nc.sync
            eng.dma_start(
                wsb[n][:].rearrange("p (j d) -> p j d", d=C)
                .rearrange("p j (i dd) -> p (j i) dd", dd=128),
                w_drams[n].ap().rearrange("j (i p) dd -> p (j i) dd", p=128))
        nc.scalar.dma_start(
            woh[:].rearrange("p (i d) -> p i d", d=C),
            woh_d.ap().rearrange("(i p) d -> p i d", p=128))
        nc.sync.dma_start(
            wol[:].rearrange("p (i d) -> p i d", d=C),
            wol_d.ap().rearrange("(i p) d -> p i d", p=128))

        def w3(t):          # [p, block, 128] view of a weight tile
            return t[:].rearrange("p (b d) -> p b d", d=128)

        def dma_mx(b, ch):
            """Fetch the 4 fp8 activation planes for (batch, chunk)."""
            tiles = {}
            for nm, d_t, eng in (("mk", mk_d, nc.sync), ("mr", mr_d, nc.scalar),
                                 ("mvh", mvh_d, nc.sync), ("mvl", mvl_d, nc.scalar)):
                tl = mxp.tile([128, NCT * TC], F8, tag=nm)
                eng.dma_start(
                    tl[:].rearrange("p (i t) -> p i t", t=TC),
                    d_t.ap()[b].rearrange("(i p) t -> p i t", p=128)
                    [:, :, ch * TC:(ch + 1) * TC])
                tiles[nm] = tl
            return tiles

        def mm_terms(ps, terms, j):
            """Accumulate DoubleRow matmul terms into psum tile ps.
            terms: list of (w_tile, act_tile); contraction pairs a in 0..3,
            weight block (j*NCT + 2a .. +2), activation blocks (2a .. 2a+2)."""
            n = len(terms) * NPAIR
            cnt = 0
            for w_t, a_t in terms:
                av = a_t[:].rearrange("p (i t) -> p i t", t=TC)
                for a in range(NPAIR):
                    nc.tensor.matmul(
                        ps[:],
                        w3(w_t)[:, j * NCT + 2 * a:j * NCT + 2 * a + 2, :],
                        av[:, 2 * a:2 * a + 2, :],
                        start=(cnt == 0), stop=(cnt == n - 1), perf_mode=DR)
                    cnt += 1

        pending_out = []

        def emit_out(ob, och, shi, slo):
            shi_v = shi[:].rearrange("p (i t) -> p i t", t=TC)
            slo_v = slo[:].rearrange("p (i t) -> p i t", t=TC)
            woh_v = woh[:].rearrange("p (i d) -> p i d", d=C)
            wol_v = wol[:].rearrange("p (i d) -> p i d", d=C)
            for m in range(NTT):
                og = osg.tile([128, C], F32, tag="ostg")
                for dh in range(2):
                    o_ps = opl.tile([128, TC], F32, tag="oproj")
                    cnt = 0
                    for wv_, sv in ((woh_v, shi_v), (wol_v, shi_v),
                                    (woh_v, slo_v)):
                        for a in range(NPAIR):
                            nc.tensor.matmul(
                                o_ps[:],
                                sv[:, 2 * a:2 * a + 2, m * 128:(m + 1) * 128],
                                wv_[:, 2 * a:2 * a + 2, dh * TC:(dh + 1) * TC],
                                start=(cnt == 0), stop=(cnt == 11), perf_mode=DR)
                            cnt += 1
                    nc.scalar.activation(og[:, dh * TC:(dh + 1) * TC], o_ps[:],
                                         AF.Copy, scale=1.0 / WS)
                trow = (och * NTT + m) * 128
                nc.sync.dma_start(out_d.ap()[ob, trow:trow + 128, :], og[:])

        mx_first = dma_mx(0, 0)

        for b in range(BL):
            carryP = carp.tile([128, NCT], F16, tag="cp")
            carryQ = carp.tile([128, NCT], F16, tag="cq")

            for ch in range(NCH):
                mx = mx_first if (b, ch) == (0, 0) else _CACHE.pop("mx_next")
                # prefetch next chunk's activations
                if ch + 1 < NCH:
                    _CACHE["mx_next"] = dma_mx(b, ch + 1)
                elif b + 1 < BL:
                    _CACHE["mx_next"] = dma_mx(b + 1, 0)

                y16 = ypl.tile([128, NCT * TC], F16, tag="y16")
                for j in range(NCT):
                    k_ps = pp.tile([128, TC], F32, tag="proj")
                    v_ps = pp.tile([128, TC], F32, tag="proj")
                    mm_terms(k_ps, [(wsb["wkh"], mx["mk"]),
                                    (wsb["wkl"], mx["mk"])], j)
                    mm_terms(v_ps, [(wsb["wvh"], mx["mvh"]),
                                    (wsb["wvl"], mx["mvh"]),
                                    (wsb["wvh"], mx["mvl"])], j)

                    ek = ekp.tile([128, TC], F16, tag="ek")
                    nc.scalar.activation(ek[:], k_ps[:], AF.Exp, scale=1.0 / WS)
                    v16 = ekp.tile([128, TC], F16, tag="v16")
                    nc.scalar.activation(v16[:], v_ps[:], AF.Copy, scale=1.0 / WS)
                    ekv = ekp.tile([128, TC], F16, tag="ekv")
                    nc.gpsimd.tensor_mul(ekv[:], ek[:], v16[:])

                    # ---- WKV scan (f16 operands, fp32 state) ----
                    pbuf = pqp.tile([128, TC + 1], F16, tag="pbuf")
                    qbuf = pqp.tile([128, TC + 1], F16, tag="qbuf")
                    if ch == 0:
                        nc.vector.memset(pbuf[:, 0:1], 0.0)
                        nc.vector.memset(qbuf[:, 0:1], 0.0)
                    else:
                        nc.scalar.copy(pbuf[:, 0:1], carryP[:, j:j + 1])
                        nc.scalar.copy(qbuf[:, 0:1], carryQ[:, j:j + 1])
                    ewj = ew16[:, j * TC:(j + 1) * TC]
                    nc.vector.tensor_tensor_scan(
                        pbuf[:, 1:TC + 1], ewj, ekv[:], pbuf[:, 0:1],
                        ALU.mult, ALU.add)
                    nc.vector.tensor_tensor_scan(
                        qbuf[:, 1:TC + 1], ewj, ek[:], qbuf[:, 0:1],
                        ALU.mult, ALU.add)
                    if ch != NCH - 1:
                        nc.scalar.copy(carryP[:, j:j + 1], pbuf[:, TC:TC + 1])
                        nc.scalar.copy(carryQ[:, j:j + 1], qbuf[:, TC:TC + 1])

                    # ---- y = (P + e^u ekv) / (Q + e^u ek) ----
                    num = ndp.tile([128, TC], F16, tag="num")
                    den = ndp.tile([128, TC], F32, tag="den")
                    eu_c = cvec[:, EU + j:EU + j + 1]
                    nc.vector.scalar_tensor_tensor(
                        num[:], ekv[:], eu_c, pbuf[:, 0:TC], ALU.mult, ALU.add)
                    nc.vector.scalar_tensor_tensor(
                        den[:], ek[:], eu_c, qbuf[:, 0:TC], ALU.mult, ALU.add)
                    nc.vector.reciprocal_approx_fast(den[:], den[:])
                    yb = y16[:, j * TC:(j + 1) * TC]
                    nc.gpsimd.tensor_mul(yb, num[:], den[:])

                while pending_out:
                    emit_out(*pending_out.pop(0))

                # ---- LN stats via PE ones-matmul: rows mu, E[y2] ----
                st_ps = sps.tile([1, 2 * TC], F32, tag="stat")
                for j in range(NCT):
                    yb = y16[:, j * TC:(j + 1) * TC]
                    ys = ysq.tile([128, TC], F16, tag="ysq")
                    nc.gpsimd.tensor_mul(ys[:], yb, yb)
                    nc.tensor.matmul(st_ps[:, 0:TC], ones16[:], yb,
                                     start=(j == 0), stop=(j == NCT - 1))
                    nc.tensor.matmul(st_ps[:, TC:2 * TC], ones16[:], ys[:],
                                     start=(j == 0), stop=(j == NCT - 1))

                # ---- r projections + tanh gate (sigmoid via tanh) ----
                th16 = thp.tile([128, NCT * TC], F16, tag="th")
                for j in range(NCT):
                    r_ps = pp.tile([128, TC], F32, tag="proj")
                    mm_terms(r_ps, [(wsb["wrh"], mx["mr"]),
                                    (wsb["wrl"], mx["mr"])], j)
                    nc.scalar.activation(th16[:, j * TC:(j + 1) * TC],
                                         r_ps[:], AF.Tanh, scale=0.5 / WS)

                # ---- LN stats post (rows on partition 0) ----
                mu16 = stp.tile([1, TC], F16, tag="mu16")
                nc.scalar.copy(mu16[:], st_ps[:, 0:TC])
                ms_t = stp.tile([1, TC], F32, tag="strow")
                nc.scalar.square(ms_t[:], st_ps[:, 0:TC])
                var_t = stp.tile([1, TC], F32, tag="strow")
                nc.vector.tensor_sub(var_t[:], st_ps[:, TC:2 * TC], ms_t[:])
                sd_t = stp.tile([1, TC], F32, tag="strow")
                nc.scalar.activation(sd_t[:], var_t[:], AF.Sqrt, bias=eps_t[:])
                nc.vector.reciprocal_approx_fast(sd_t[:], sd_t[:])
                rs16 = stp.tile([1, TC], F16, tag="rs16")
                nc.vector.tensor_scalar(rs16[:], sd_t[:], 1.0, None, ALU.mult)
                rsb = stp.tile([128, TC], F16, tag="bcast")
                nc.gpsimd.partition_broadcast(rsb[:], rs16[:])
                mub = stp.tile([128, TC], F16, tag="bcast")
                nc.gpsimd.partition_broadcast(mub[:], mu16[:])

                # ---- gate: sry = (th+1) * (LW2*(y-mu)*rstd + LB2) ----
                sry_h = syp.tile([128, NCT * TC], F8, tag="sryh")
                sry_l = syp.tile([128, NCT * TC], F8, tag="sryl")
                for j in range(NCT):
                    s = slice(j * TC, (j + 1) * TC)
                    ya = gtp.tile([128, TC], F16, tag="gt")
                    nc.gpsimd.tensor_sub(ya[:], y16[:, s], mub[:])
                    ybt = gtp.tile([128, TC], F16, tag="gt")
                    nc.vector.tensor_mul(ybt[:], ya[:], rsb[:])
                    yct = gtp.tile([128, TC], F16, tag="gt")
                    nc.scalar.activation(
                        yct[:], ybt[:], AF.Identity,
                        bias=cvec[:, LB2 + j:LB2 + j + 1],
                        scale=cvec[:, LW2 + j:LW2 + j + 1])
                    sry = gtp.tile([128, TC], F16, tag="gt")
                    nc.vector.scalar_tensor_tensor(
                        sry[:], th16[:, s], one_t[:], yct[:],
                        ALU.add, ALU.mult)
                    nc.scalar.activation(sry_h[:, s], sry[:], AF.Copy)
                    nc.vector.tensor_tensor(sry_l[:, s], sry[:], sry_h[:, s],
                                            ALU.subtract)

                pending_out.append((b, ch, sry_h, sry_l))
                if b == BL - 1 and ch == NCH - 1:
                    while pending_out:
                        emit_out(*pending_out.pop(0))

    nc.compile()
    return nc


def _pack(v):
    return np.ascontiguousarray(v.reshape(NCT, 128).T.astype(np.float32))


def _q8(a):
    return a.astype(E4)


def _qshift(x):
    """xx[b,t,c]: 4 channel groups shifted by (w-1, w+1, h-1, h+1), zeros
    at the spatial boundary. x is [B, T, C] with T = 32*32 row-major."""
    Bq, Tq, Cq = x.shape
    g = Cq // 4
    xs = x.reshape(Bq, 32, 32, Cq)
    xx = np.zeros_like(xs)
    xx[:, :, 1:, 0:g] = xs[:, :, :-1, 0:g]          # from w-1
    xx[:, :, :-1, g:2 * g] = xs[:, :, 1:, g:2 * g]  # from w+1
    xx[:, 1:, :, 2 * g:3 * g] = xs[:, :-1, :, 2 * g:3 * g]  # from h-1
    xx[:, :-1, :, 3 * g:] = xs[:, 1:, :, 3 * g:]    # from h+1
    return xx.reshape(Bq, Tq, Cq)


def kernel(x, Wk, Wv, Wr, Wo, ln_w, ln_b, spatial_decay, spatial_first,
           mix_k, mix_v, mix_r, H, W):
    x = np.asarray(x, dtype=np.float32)
    assert int(H) == 32 and int(W) == 32 and x.shape == (B, T, C)

    if "nc" not in _CACHE:
        _CACHE["nc"] = build()
    nc = _CACHE["nc"]

    w_eff = -np.exp(np.asarray(spatial_decay, np.float64) / T)
    u_eff = np.asarray(spatial_first, np.float64) / T
    cvec = np.concatenate([
        _pack(np.exp(w_eff).astype(np.float32)),
        _pack(np.exp(u_eff).astype(np.float32)),
        _pack(np.asarray(ln_w, np.float32) * 0.5),
        _pack(np.asarray(ln_b, np.float32) * 0.5),
    ], axis=1)

    # ---- host: mixed activations, quantized to fp8 (channel-major) ----
    xx = _qshift(x)
    mk = np.asarray(mix_k, np.float32)
    mv = np.asarray(mix_v, np.float32)
    mr = np.asarray(mix_r, np.float32)
    xk = x * mk + xx * (1.0 - mk)
    xv = x * mv + xx * (1.0 - mv)
    xr = x * mr + xx * (1.0 - mr)

    def _cm(a):                      # [B,T,C] -> [B,C,T]
        return np.ascontiguousarray(a.transpose(0, 2, 1))
    mk8 = _q8(_cm(xk))
    mr8 = _q8(_cm(xr))
    xv_cm = _cm(xv)
    mvh8 = _q8(xv_cm)
    mvl8 = _q8(xv_cm - mvh8.astype(np.float32))

    # ---- host: fp8 hi/lo weights at shared x64 scale ----
    def _split(Wm):
        wt = np.asarray(Wm, np.float32).T * WS           # [C, D]
        hi = _q8(wt)
        lo = _q8(wt - hi.astype(np.float32))
        return hi, lo

    def _jmajor(w8):                 # [C, D] fp8 -> [j, c, dd]
        return np.ascontiguousarray(
            w8.reshape(C, NCT, 128).transpose(1, 0, 2))
    wkh, wkl = _split(Wk)
    wvh, wvl = _split(Wv)
    wrh, wrl = _split(Wr)
    woh, wol = _split(Wo)

    in_maps = []
    for c in range(NCORES):
        sl = slice(c * BL, (c + 1) * BL)
        in_maps.append({
            "mk": mk8[sl], "mr": mr8[sl], "mvh": mvh8[sl], "mvl": mvl8[sl],
            "wkh": _jmajor(wkh), "wkl": _jmajor(wkl),
            "wvh": _jmajor(wvh), "wvl": _jmajor(wvl),
            "wrh": _jmajor(wrh), "wrl": _jmajor(wrl),
            "woh": np.ascontiguousarray(woh), "wol": np.ascontiguousarray(wol),
            "cvec": cvec,
        })
    last_err = None
    for _attempt in range(3):
        try:
            res = run_bass_kernel_spmd(nc, in_maps,
                                       core_ids=list(range(NCORES)))
            break
        except Exception as e:  # transient device wedge: retry
            last_err = e
            import time as _time
            _time.sleep(2.0)
    else:
        raise last_err
    out = np.concatenate([res.results[c]["out"] for c in range(NCORES)], axis=0)
    return out.astype(np.float32)
